# revision 2
# baseline (speedup 1.0000x reference)
import os

_flags = os.environ.get("NEURON_CC_FLAGS", "")
if "--auto-cast" not in _flags:
    os.environ["NEURON_CC_FLAGS"] = (_flags + " --auto-cast none").strip()

import numpy as np
import jax
import jax.numpy as jnp
from jax import lax

EPS = 1e-5
N_CORES = 8


def _sign(x):
    return jnp.where(x >= 0, 1.0, -1.0).astype(x.dtype)


def _bn_thresh(h, gamma, beta, mean, var, shape):
    inv = (gamma / jnp.sqrt(var + EPS)).reshape(shape)
    return (h - mean.reshape(shape)) * inv + beta.reshape(shape)


def _conv_rep(x, wb):
    xp = jnp.pad(x, ((0, 0), (0, 0), (1, 1), (1, 1)), mode='edge')
    return lax.conv_general_dilated(xp, wb, (1, 1), 'VALID',
                                    dimension_numbers=('NCHW', 'OIHW', 'NCHW'))


def _maxpool2(x):
    return lax.reduce_window(x, -jnp.inf, lax.max, (1, 1, 2, 2), (1, 1, 2, 2), 'VALID')


def _forward(x, conv1_w, bn1_gamma, bn1_beta, bn1_mean, bn1_var,
             conv2_w, bn2_gamma, bn2_beta, bn2_mean, bn2_var,
             fc1_w, bn3_gamma, bn3_beta, bn3_mean, bn3_var,
             fc2_w, scale):
    c4 = (1, -1, 1, 1)
    c2 = (1, -1)
    h = _conv_rep(x, _sign(conv1_w))
    h = _sign(jnp.clip(_bn_thresh(h, bn1_gamma, bn1_beta, bn1_mean, bn1_var, c4), -1.0, 1.0))
    h = _maxpool2(h)
    h = _conv_rep(h, _sign(conv2_w))
    h = _sign(jnp.clip(_bn_thresh(h, bn2_gamma, bn2_beta, bn2_mean, bn2_var, c4), -1.0, 1.0))
    h = _maxpool2(h)
    h = h.reshape(h.shape[0], -1)
    h = h @ _sign(fc1_w).T
    h = _sign(jnp.clip(_bn_thresh(h, bn3_gamma, bn3_beta, bn3_mean, bn3_var, c2), -1.0, 1.0))
    h = h @ _sign(fc2_w).T
    return h * scale


_pfwd = jax.pmap(_forward, in_axes=(0,) + (None,) * 17)


def kernel(**inputs):
    x = np.asarray(inputs['x'])
    B = x.shape[0]
    xs = x.reshape(N_CORES, B // N_CORES, *x.shape[1:])
    names = ['conv1_w', 'bn1_gamma', 'bn1_beta', 'bn1_mean', 'bn1_var',
             'conv2_w', 'bn2_gamma', 'bn2_beta', 'bn2_mean', 'bn2_var',
             'fc1_w', 'bn3_gamma', 'bn3_beta', 'bn3_mean', 'bn3_var',
             'fc2_w', 'scale']
    rest = [np.asarray(inputs[n]) for n in names]
    out = _pfwd(xs, *rest)
    out = np.asarray(out)
    return out.reshape(B, out.shape[-1]).astype(np.float32)


# revision 3
# speedup vs baseline: 2.1575x; 2.1575x over previous
import os

_flags = os.environ.get("NEURON_CC_FLAGS", "")
if "--auto-cast" not in _flags:
    os.environ["NEURON_CC_FLAGS"] = (_flags + " --auto-cast none").strip()

import numpy as np
import jax
import jax.numpy as jnp
from jax import lax

EPS = 1e-5
N_CORES = 8


def _sign(x):
    return jnp.where(x >= 0, 1.0, -1.0).astype(x.dtype)


def _bn_thresh(h, gamma, beta, mean, var, shape):
    inv = (gamma / jnp.sqrt(var + EPS)).reshape(shape)
    return (h - mean.reshape(shape)) * inv + beta.reshape(shape)


def _conv_rep(x, wb):
    xp = jnp.pad(x, ((0, 0), (0, 0), (1, 1), (1, 1)), mode='edge')
    return lax.conv_general_dilated(xp, wb, (1, 1), 'VALID',
                                    dimension_numbers=('NCHW', 'OIHW', 'NCHW'))


def _maxpool2(x):
    return lax.reduce_window(x, -jnp.inf, lax.max, (1, 1, 2, 2), (1, 1, 2, 2), 'VALID')


def _forward(x, w1b, bn1_gamma, bn1_beta, bn1_mean, bn1_var,
             w2b, bn2_gamma, bn2_beta, bn2_mean, bn2_var,
             w3bT, bn3_gamma, bn3_beta, bn3_mean, bn3_var,
             w4bT, scale):
    c4 = (1, -1, 1, 1)
    c2 = (1, -1)
    # conv1: real-valued x -> exact fp32 conv with +/-1 weights
    h = _conv_rep(x, w1b)
    h = _sign(jnp.clip(_bn_thresh(h, bn1_gamma, bn1_beta, bn1_mean, bn1_var, c4), -1.0, 1.0))
    h = _maxpool2(h)
    # conv2: +/-1 activations x +/-1 weights -> bf16 inputs are exact,
    # fp32 accumulation of +/-1 products is exact integers
    hb = h.astype(jnp.bfloat16)
    xp = jnp.pad(hb, ((0, 0), (0, 0), (1, 1), (1, 1)), mode='edge')
    h = lax.conv_general_dilated(xp, w2b, (1, 1), 'VALID',
                                 dimension_numbers=('NCHW', 'OIHW', 'NCHW'),
                                 preferred_element_type=jnp.float32)
    h = _sign(jnp.clip(_bn_thresh(h, bn2_gamma, bn2_beta, bn2_mean, bn2_var, c4), -1.0, 1.0))
    h = _maxpool2(h)
    h = h.reshape(h.shape[0], -1).astype(jnp.bfloat16)
    h = lax.dot(h, w3bT, preferred_element_type=jnp.float32)
    h = _sign(jnp.clip(_bn_thresh(h, bn3_gamma, bn3_beta, bn3_mean, bn3_var, c2), -1.0, 1.0))
    h = lax.dot(h.astype(jnp.bfloat16), w4bT, preferred_element_type=jnp.float32)
    return h * scale


_pfwd = jax.pmap(_forward, in_axes=(0,) + (None,) * 17)


def _npsign(w):
    return np.where(w >= 0, np.float32(1.0), np.float32(-1.0))


def kernel(**inputs):
    x = np.asarray(inputs['x'])
    B = x.shape[0]
    xs = x.reshape(N_CORES, B // N_CORES, *x.shape[1:])
    import ml_dtypes
    bf16 = ml_dtypes.bfloat16
    w1b = _npsign(np.asarray(inputs['conv1_w'])).astype(np.float32)
    w2b = _npsign(np.asarray(inputs['conv2_w'])).astype(bf16)
    w3bT = np.ascontiguousarray(_npsign(np.asarray(inputs['fc1_w'])).T).astype(bf16)
    w4bT = np.ascontiguousarray(_npsign(np.asarray(inputs['fc2_w'])).T).astype(bf16)
    names = ['bn1_gamma', 'bn1_beta', 'bn1_mean', 'bn1_var',
             'bn2_gamma', 'bn2_beta', 'bn2_mean', 'bn2_var',
             'bn3_gamma', 'bn3_beta', 'bn3_mean', 'bn3_var', 'scale']
    bn = {n: np.asarray(inputs[n]) for n in names}
    out = _pfwd(xs, w1b, bn['bn1_gamma'], bn['bn1_beta'], bn['bn1_mean'], bn['bn1_var'],
                w2b, bn['bn2_gamma'], bn['bn2_beta'], bn['bn2_mean'], bn['bn2_var'],
                w3bT, bn['bn3_gamma'], bn['bn3_beta'], bn['bn3_mean'], bn['bn3_var'],
                w4bT, bn['scale'])
    out = np.asarray(out)
    return out.reshape(B, out.shape[-1]).astype(np.float32)
